# revision 13
# baseline (speedup 1.0000x reference)
"""DenseWrite MoE-routing kernel for 8x Trainium2 NeuronCores.

Computation (reference):
    Un = U / ||U||_col          (col norm over D+1=513 axis, per (m, b))
    U_active = Un[topk][:, :, :D, :]                 # (N, K, D, B)
    writes   = einsum('nkdb,nkb->nd', U_active, h)   # (N, D)
    h_recon  = einsum('nkdb,nd->nkb', U_active, writes)
    loss     = mean((h_recon - h)^2)

Strategy: data-parallel over tokens (1024 per core).  All routing is done
ON the TensorEngine with one-hot matmuls (no gathers/scatters):

 -  S2[m, 16*t + b] = sum_k 1[e(t,k)=m] * h[t,k,b]   built per 128-pair chunk
    as  P_onehot.T @ blockdiag(h),  where blockdiag(h) = E .* broadcast(h)
    costs one DVE multiply per chunk.  A second accumulating matmul writes a
    b-shifted duplicate into partitions 64:128 so the writes matmul can
    contract over the full 128 partitions (pairs of b-planes).
 -  writes^T = sum_b Ustack_b.T @ S2_pair_view_b    (contraction = 2x64 experts)
 -  loss: sum(R .* S) == ||writes||^2 exactly, and
    sum(c * R^2) via R_j = Un_j^T @ writes^T, squared on ScalarE, then a
    static 0/1 "sum over b" matmul + count-weighted reduce.  Counts come from
    the same one-hot matmuls (rhs = static token indicator).
All matmuls run in float32r (measured rel. err ~1.5e-4, full PE rate).
"""

import sys

sys.path.insert(0, "/opt/trn_rl_repo")

import numpy as np
from concourse import bacc, bass, mybir, tile
from concourse.bass_utils import run_bass_kernel_spmd

f32 = mybir.dt.float32
f32r = mybir.dt.float32r
AF = mybir.ActivationFunctionType
ALU = mybir.AluOpType

# problem constants
N, K, B, M, D = 8192, 4, 16, 64, 512
NC = 8               # cores
NS = N // NC         # tokens per core (1024)
NPAIR = NS * K       # pairs per core (4096)
CH = NPAIR // 128    # 128-pair chunks per core (32)
TPC = 128 // K       # tokens per chunk (32)
UF = (D + 1) * B     # 8208, free size of U rows

# const layout inside the single "inp" DRAM tensor (free offsets)
OFF_HP = 0            # (128, 528)  h pairs (+16 pad)
OFF_EP = 528          # (128, 32)   expert ids (fp32)
OFF_E = 560           # (128, 512)  block-diagonal expansion mask
OFF_TT = 1072         # (128, 32)   token indicator (pairs -> token)
OFF_CV = 1104         # (128, 64)   iota row 0..63 (pre-broadcast)
OFF_ID = 1168         # (128, 128)  identity
OFF_KS = 1296         # (128, 512)  8x (128,64) "sum over b" one-hots
INP_F = 1808


def build_nc():
    nc = bacc.Bacc("TRN2", target_bir_lowering=False, debug=False)
    inp = nc.dram_tensor("inp", [128, INP_F], f32, kind="ExternalInput")
    uD = nc.dram_tensor("u", [M, UF], f32, kind="ExternalInput")
    wtD = nc.dram_tensor("wt", [128, 4 * NS + 8], f32, kind="ExternalOutput")

    with tile.TileContext(nc) as tc:
        with (
            tc.tile_pool(name="persist", bufs=1) as pp,
            tc.tile_pool(name="uprep", bufs=1) as up,
            tc.tile_pool(name="work", bufs=3) as wp,
        ):
            # ---- persistent tiles
            inpS = pp.tile([128, INP_F], f32, tag="inp")
            uS = up.tile([128, UF], f32, tag="u")
            s2d = pp.tile([128, 16 * NS], f32r, tag="s2d")
            ustack = [pp.tile([128, D], f32r, tag=f"ust{i}", name=f"ust{i}") for i in range(8)]
            unt = [pp.tile([128, M * B], f32r, tag=f"unt{i}", name=f"unt{i}") for i in range(4)]
            wtT = pp.tile([128, 4 * NS + 8], f32r, tag="wtT")
            cntS = pp.tile([M, NS], f32, tag="cnt")
            nsq = pp.tile([128, B], f32, tag="nsq")
            rinv = pp.tile([128, B], f32, tag="rinv")
            nrt = pp.tile([128, B], f32, tag="nrt")
            idR = pp.tile([128, 128], f32r, tag="idr")
            ksR = pp.tile([128, 512], f32r, tag="ksr")
            ttR = pp.tile([128, TPC], f32r, tag="ttr")
            wacc = pp.tile([128, 4], f32, tag="wacc")
            wacc2 = pp.tile([128, 1], f32, tag="wacc2")
            hacc = pp.tile([128, 1], f32, tag="hacc")
            scr = pp.tile([128, 1024], f32, tag="scr")
            qacc = pp.tile([M, 2], f32, tag="qacc")
            qacc2 = pp.tile([M, 1], f32, tag="qacc2")
            qtot = pp.tile([1, 1], f32, tag="qtot")
            wtot = pp.tile([1, 1], f32, tag="wtot")
            htot = pp.tile([1, 1], f32, tag="htot")

            # ---- input DMAs
            nc.sync.dma_start(out=inpS[:], in_=inp[:])
            nc.sync.dma_start(out=uS[:M, :], in_=uD[:])
            # duplicate U onto partitions 64:128 (for the b-pair stacking)
            nc.sync.dma_start(out=uS[M:128, :], in_=uS[:M, :])

            fdve = pp.tile([1, 4], f32, tag="fdve")
            nc.vector.tensor_copy(out=fdve[:, 0:1], in_=inpS[0:1, 0:1])
            nc.vector.tensor_copy(out=fdve[:, 1:2], in_=uS[0:1, 0:1])
            nc.vector.tensor_copy(out=fdve[:, 2:3], in_=uS[M : M + 1, 0:1])

            hp = inpS[:, OFF_HP : OFF_HP + 528]
            ep = inpS[:, OFF_EP : OFF_EP + CH]
            eM = inpS[:, OFF_E : OFF_E + 512]
            cv = inpS[:, OFF_CV : OFF_CV + 64]

            # rounded (f32r) copies of matmul constants
            nc.vector.tensor_copy(out=idR[:], in_=inpS[:, OFF_ID : OFF_ID + 128])
            nc.vector.tensor_copy(out=ksR[:], in_=inpS[:, OFF_KS : OFF_KS + 512])
            nc.vector.tensor_copy(out=ttR[:], in_=inpS[:, OFF_TT : OFF_TT + TPC])

            # ---- U normalization -> ustack (experts on partitions, dup halves)
            uview = uS[:].rearrange("p (d b) -> p d b", b=B)
            for bb in range(B):
                uscr = up.tile([128, D + 1], f32, tag="uscr", bufs=2)
                nc.vector.scalar_tensor_tensor(
                    out=uscr[:],
                    in0=uview[:, :, bb],
                    scalar=1.0,
                    in1=uview[:, :, bb],
                    op0=ALU.mult,
                    op1=ALU.mult,
                    accum_out=nsq[:, bb : bb + 1],
                )
            nc.scalar.activation(out=nrt[:], in_=nsq[:], func=AF.Sqrt)
            nc.vector.reciprocal(out=rinv[:], in_=nrt[:])
            for bb in range(8):
                nc.vector.tensor_scalar(
                    out=ustack[bb][:M, :],
                    in0=uview[:M, :D, bb],
                    scalar1=rinv[:M, bb : bb + 1],
                    scalar2=None,
                    op0=ALU.mult,
                )
                nc.vector.tensor_scalar(
                    out=ustack[bb][M:128, :],
                    in0=uview[M:128, :D, bb + 8],
                    scalar1=rinv[M:128, bb + 8 : bb + 9],
                    scalar2=None,
                    op0=ALU.mult,
                )

            # ================= phase A (+ U transposes), own PSUM scope
            with (
                tc.tile_pool(name="psA", bufs=2, space="PSUM") as ppsA,
                tc.tile_pool(name="psB", bufs=1, space="PSUM") as ppsB,
            ):
                # ---- Un^T tiles (d on partitions) via PE transpose
                for dch in range(4):
                    psT = ppsA.tile([128, 1024], f32r, space="PSUM", tag="psT")
                    for ab in range(B):
                        src = ustack[ab % 8]
                        rows = src[:M, :] if ab < 8 else src[M:128, :]
                        ident = idR[:M, :M] if ab < 8 else idR[M:128, M:128]
                        nc.tensor.transpose(
                            out=psT[:, 64 * ab : 64 * ab + 64],
                            in_=rows[:, 128 * dch : 128 * dch + 128],
                            identity=ident,
                        )
                    nc.vector.tensor_copy(
                        out=unt[dch][:].rearrange("p (m b) -> p b m", b=B),
                        in_=psT[:].rearrange("p (b m) -> p b m", m=M),
                    )

                # ---- build S2 (+shifted dup) and counts
                psCnt = ppsB.tile([M, NS], f32, space="PSUM", tag="psCnt")
                hview = hp.rearrange("p (c b) -> p c b", b=B)
                p2cyc = [
                    pp.tile([128, 192], f32r, tag=f"p2_{i}", name=f"p2_{i}")
                    for i in range(3)
                ]
                for t in p2cyc:
                    nc.vector.memset(t[:, 0:64].bitcast(f32), 0.0)
                    nc.vector.memset(t[:, 128:192].bitcast(f32), 0.0)
                for c in range(CH):
                    p2 = p2cyc[c % 3]
                    nc.vector.tensor_scalar(
                        out=p2[:, 64:128],
                        in0=cv,
                        scalar1=ep[:, c : c + 1],
                        scalar2=None,
                        op0=ALU.is_equal,
                    )
                    dc = wp.tile([128, 512], f32r, tag="dc")
                    dsh = wp.tile([128, 512], f32r, tag="dsh")
                    nc.vector.tensor_tensor(
                        out=dc[:].rearrange("p (t b) -> p t b", b=B),
                        in0=eM.rearrange("p (t b) -> p t b", b=B),
                        in1=hview[:, c, :][:, None, :].to_broadcast((128, TPC, B)),
                        op=ALU.mult,
                    )
                    nc.vector.tensor_tensor(
                        out=dsh[:].rearrange("p (t b) -> p t b", b=B),
                        in0=eM.rearrange("p (t b) -> p t b", b=B),
                        in1=hp[:, 16 * c + 8 : 16 * c + 24][:, None, :].to_broadcast(
                            (128, TPC, B)
                        ),
                        op=ALU.mult,
                    )
                    psA = ppsA.tile([128, 512], f32, space="PSUM", tag="psA")
                    nc.tensor.matmul(
                        psA[:], lhsT=p2[:, 64:192], rhs=dc[:], start=True, stop=False
                    )
                    nc.tensor.matmul(
                        psA[:], lhsT=p2[:, 0:128], rhs=dsh[:], start=False, stop=True
                    )
                    nc.tensor.matmul(
                        psCnt[:, TPC * c : TPC * (c + 1)],
                        lhsT=p2[:, 64:128],
                        rhs=ttR[:],
                        start=True,
                        stop=True,
                    )
                    nc.vector.tensor_copy(out=s2d[:, 512 * c : 512 * (c + 1)], in_=psA[:])

                nc.vector.tensor_copy(out=cntS[:], in_=psCnt[:])

            # ================= writes phase (needs all 8 PSUM banks)
            with tc.tile_pool(name="psW", bufs=8, space="PSUM") as ppsW:
                s2view = s2d[:].rearrange("p (t b) -> p t b", b=B)
                for dch in range(4):
                    for nch in range(2):
                        psW = ppsW.tile([128, 512], f32, space="PSUM", tag="psW")
                        for bb in range(8):
                            nc.tensor.matmul(
                                psW[:],
                                lhsT=ustack[bb][:, 128 * dch : 128 * (dch + 1)],
                                rhs=s2view[:, 512 * nch : 512 * (nch + 1), bb],
                                start=(bb == 0),
                                stop=(bb == 7),
                            )
                        nc.vector.tensor_copy(
                            out=wtT[
                                :, 1024 * dch + 512 * nch : 1024 * dch + 512 * (nch + 1)
                            ],
                            in_=psW[:],
                        )
                # ||writes||^2 and sum(h^2) (DVE fused square+reduce)
                for dch in range(4):
                    sl = wtT[:, 1024 * dch : 1024 * (dch + 1)]
                    nc.vector.scalar_tensor_tensor(
                        out=scr[:],
                        in0=sl,
                        scalar=1.0,
                        in1=sl,
                        op0=ALU.mult,
                        op1=ALU.mult,
                        accum_out=wacc[:, dch : dch + 1],
                    )
                nc.vector.scalar_tensor_tensor(
                    out=scr[:, :512],
                    in0=inpS[:, OFF_HP : OFF_HP + 512],
                    scalar=1.0,
                    in1=inpS[:, OFF_HP : OFF_HP + 512],
                    op0=ALU.mult,
                    op1=ALU.mult,
                    accum_out=hacc[:],
                )
                nc.vector.tensor_reduce(
                    out=wacc2[:], in_=wacc[:], axis=mybir.AxisListType.X, op=ALU.add
                )

            # ================= recon/loss phase
            with (
                tc.tile_pool(name="psR", bufs=2, space="PSUM") as ppsR,
                tc.tile_pool(name="psQ", bufs=2, space="PSUM") as ppsQ,
            ):
                psQ = [
                    ppsQ.tile([M, 512], f32, space="PSUM", tag=f"psQ{i}", name=f"psQ{i}")
                    for i in range(2)
                ]
                for nch in range(2):
                    for j in range(8):
                        psR = ppsR.tile([128, 512], f32, space="PSUM", tag="psR")
                        nc.vector.memset(psR[0:1, 0:1], 0.0)
                        for dch in range(4):
                            nc.tensor.matmul(
                                psR[:],
                                lhsT=unt[dch][:, 128 * j : 128 * (j + 1)],
                                rhs=wtT[
                                    :,
                                    1024 * dch + 512 * nch : 1024 * dch + 512 * (nch + 1),
                                ],
                                start=(dch == 0),
                                stop=(dch == 3),
                            )
                        sqR = wp.tile([128, 512], f32r, tag="sqR")
                        nc.scalar.activation(out=sqR[:], in_=psR[:], func=AF.Square)
                        nc.tensor.matmul(
                            psQ[nch][:],
                            lhsT=ksR[:, 64 * j : 64 * (j + 1)],
                            rhs=sqR[:],
                            start=(j == 0),
                            stop=(j == 7),
                        )
                for nch in range(2):
                    nc.vector.scalar_tensor_tensor(
                        out=scr[:M, :512],
                        in0=psQ[nch][:],
                        scalar=1.0,
                        in1=cntS[:, 512 * nch : 512 * (nch + 1)],
                        op0=ALU.mult,
                        op1=ALU.mult,
                        accum_out=qacc[:, nch : nch + 1],
                    )
                nc.vector.tensor_reduce(
                    out=qacc2[:], in_=qacc[:], axis=mybir.AxisListType.X, op=ALU.add
                )
                # cross-partition reduce -> scalars (single packed op)
                acc3 = pp.tile([128, 4], f32, tag="acc3", name="acc3")
                tot3 = pp.tile([1, 4], f32, tag="tot3", name="tot3")
                nc.vector.memset(acc3[:], 0.0)
                nc.vector.tensor_copy(out=acc3[:M, 0:1], in_=qacc2[:])
                nc.vector.tensor_copy(out=acc3[:, 1:2], in_=wacc2[:])
                nc.vector.tensor_copy(out=acc3[:, 2:3], in_=hacc[:])
                nc.gpsimd.tensor_reduce(
                    out=tot3[:], in_=acc3[:], axis=mybir.AxisListType.C, op=ALU.add
                )
                nc.vector.memset(wtT[:, 4 * NS : 4 * NS + 8].bitcast(f32), 0.0)
                nc.vector.tensor_copy(
                    out=wtT[0:1, 4 * NS : 4 * NS + 4], in_=tot3[:]
                )
                nc.sync.dma_start(out=wtD[:], in_=wtT[:].bitcast(f32))

    nc.compile()
    return nc


def _host_constants():
    p = np.arange(128)
    t = p // K
    E = np.zeros((128, 512), np.float32)
    for b in range(B):
        E[p, 16 * t + b] = 1.0
    TT = np.zeros((128, TPC), np.float32)
    TT[p, t] = 1.0
    CV = np.broadcast_to(np.arange(64, dtype=np.float32), (128, 64)).copy()
    ID = np.eye(128, dtype=np.float32)
    KS = np.zeros((128, 512), np.float32)
    q = np.arange(128)
    for j in range(8):
        KS[q, 64 * j + 8 * j + q // 16] = 1.0
    return E, TT, CV, ID, KS


_NC_CACHE = {}


def kernel(h_sparse, topk_idxs, U):
    h_sparse = np.asarray(h_sparse, dtype=np.float32)
    topk_idxs = np.asarray(topk_idxs)
    U = np.asarray(U, dtype=np.float32)

    if "nc" not in _NC_CACHE:
        _NC_CACHE["nc"] = build_nc()
    nc = _NC_CACHE["nc"]

    E, TT, CV, ID, KS = _host_constants()
    u_in = np.ascontiguousarray(U.reshape(M, UF))

    in_maps = []
    for i in range(NC):
        sl = slice(NS * i, NS * (i + 1))
        hs = h_sparse[sl].reshape(NPAIR, B)
        hp = np.zeros((128, 528), np.float32)
        hp[:, :512] = hs.reshape(CH, 128, B).transpose(1, 0, 2).reshape(128, 512)
        ep = (
            topk_idxs[sl].reshape(NPAIR).astype(np.float32).reshape(CH, 128).T.copy()
        )
        inp = np.zeros((128, INP_F), np.float32)
        inp[:, OFF_HP : OFF_HP + 528] = hp
        inp[:, OFF_EP : OFF_EP + CH] = ep
        inp[:, OFF_E : OFF_E + 512] = E
        inp[:, OFF_TT : OFF_TT + TPC] = TT
        inp[:, OFF_CV : OFF_CV + 64] = CV
        inp[:, OFF_ID : OFF_ID + 128] = ID
        inp[:, OFF_KS : OFF_KS + 512] = KS
        in_maps.append({"inp": inp, "u": u_in})

    br = run_bass_kernel_spmd(
        nc, in_maps, list(range(NC)), trace=bool(_NC_CACHE.get("trace"))
    )
    if _NC_CACHE.get("trace"):
        _NC_CACHE["last_exec_ns"] = br.exec_time_ns
        _NC_CACHE["last_profile"] = br.profile_json
    res = br.results

    writes = np.empty((N, D), np.float32)
    cr2 = w2 = h2 = 0.0
    for i in range(NC):
        wtfull = res[i]["wt"]  # (128, 4*NS + 8)
        wt2 = wtfull[:, : 4 * NS]
        writes[NS * i : NS * (i + 1)] = (
            wt2.reshape(128, 4, NS).transpose(2, 1, 0).reshape(NS, D)
        )
        plv = wtfull[0, 4 * NS : 4 * NS + 4]
        cr2 += float(plv[0])
        w2 += float(plv[1])
        h2 += float(plv[2])
    loss = (cr2 - 2.0 * w2 + h2) / float(N * K * B)
    return writes, np.float32(loss)


# revision 14
# speedup vs baseline: 1.0517x; 1.0517x over previous
"""DenseWrite MoE-routing kernel for 8x Trainium2 NeuronCores.

Computation (reference):
    Un = U / ||U||_col          (col norm over D+1=513 axis, per (m, b))
    U_active = Un[topk][:, :, :D, :]                 # (N, K, D, B)
    writes   = einsum('nkdb,nkb->nd', U_active, h)   # (N, D)
    h_recon  = einsum('nkdb,nd->nkb', U_active, writes)
    loss     = mean((h_recon - h)^2)

Strategy: data-parallel over tokens (1024 per core).  All routing is done
ON the TensorEngine with one-hot matmuls (no gathers/scatters):

 -  S2[m, 16*t + b] = sum_k 1[e(t,k)=m] * h[t,k,b]   built per 128-pair chunk
    as  P_onehot.T @ blockdiag(h),  where blockdiag(h) = E .* broadcast(h)
    costs one DVE multiply per chunk.  A second accumulating matmul writes a
    b-shifted duplicate into partitions 64:128 so the writes matmul can
    contract over the full 128 partitions (pairs of b-planes).
 -  writes^T = sum_b Ustack_b.T @ S2_pair_view_b    (contraction = 2x64 experts)
 -  loss: sum(R .* S) == ||writes||^2 exactly, and
    sum(c * R^2) via R_j = Un_j^T @ writes^T, squared on ScalarE, then a
    static 0/1 "sum over b" matmul + count-weighted reduce.  Counts come from
    the same one-hot matmuls (rhs = static token indicator).
All matmuls run in float32r (measured rel. err ~1.5e-4, full PE rate).
"""

import sys

sys.path.insert(0, "/opt/trn_rl_repo")

import numpy as np
from concourse import bacc, bass, mybir, tile
from concourse.bass_utils import run_bass_kernel_spmd

f32 = mybir.dt.float32
f32r = mybir.dt.float32r
AF = mybir.ActivationFunctionType
ALU = mybir.AluOpType

# problem constants
N, K, B, M, D = 8192, 4, 16, 64, 512
NC = 8               # cores
NS = N // NC         # tokens per core (1024)
NPAIR = NS * K       # pairs per core (4096)
CH = NPAIR // 128    # 128-pair chunks per core (32)
TPC = 128 // K       # tokens per chunk (32)
UF = (D + 1) * B     # 8208, free size of U rows

# const layout inside the single "inp" DRAM tensor (free offsets)
OFF_HP = 0            # (128, 528)  h pairs (+16 pad)
OFF_EP = 528          # (128, 32)   expert ids (fp32)
OFF_E = 560           # (128, 512)  block-diagonal expansion mask
OFF_TT = 1072         # (128, 32)   token indicator (pairs -> token)
OFF_CV = 1104         # (128, 64)   iota row 0..63 (pre-broadcast)
OFF_ID = 1168         # (128, 128)  identity
OFF_KS = 1296         # (128, 512)  8x (128,64) "sum over b" one-hots
INP_F = 1808


def build_nc():
    nc = bacc.Bacc("TRN2", target_bir_lowering=False, debug=False)
    inp = nc.dram_tensor("inp", [128, INP_F], f32, kind="ExternalInput")
    uD = nc.dram_tensor("u", [M, UF], f32, kind="ExternalInput")
    wtD = nc.dram_tensor("wt", [128, 4 * NS], f32, kind="ExternalOutput")
    plD = nc.dram_tensor("pl", [1, 8], f32, kind="ExternalOutput")

    with tile.TileContext(nc) as tc:
        with (
            tc.tile_pool(name="persist", bufs=1) as pp,
            tc.tile_pool(name="uprep", bufs=1) as up,
            tc.tile_pool(name="work", bufs=3) as wp,
        ):
            # ---- persistent tiles
            inpS = pp.tile([128, INP_F], f32, tag="inp")
            uS = up.tile([128, UF], f32, tag="u")
            s2d = pp.tile([128, 16 * NS], f32r, tag="s2d")
            ustack = [pp.tile([128, D], f32r, tag=f"ust{i}", name=f"ust{i}") for i in range(8)]
            unt = [pp.tile([128, M * B], f32r, tag=f"unt{i}", name=f"unt{i}") for i in range(4)]
            wtT = pp.tile([128, 4 * NS], f32r, tag="wtT")
            pl = pp.tile([1, 8], f32, tag="pl")
            cntS = pp.tile([M, NS], f32, tag="cnt")
            nsq = pp.tile([128, B], f32, tag="nsq")
            rinv = pp.tile([128, B], f32, tag="rinv")
            nrt = pp.tile([128, B], f32, tag="nrt")
            idR = pp.tile([128, 128], f32r, tag="idr")
            ksR = pp.tile([128, 512], f32r, tag="ksr")
            ttR = pp.tile([128, TPC], f32r, tag="ttr")
            wacc = pp.tile([128, 4], f32, tag="wacc")
            wacc2 = pp.tile([128, 1], f32, tag="wacc2")
            hacc = pp.tile([128, 1], f32, tag="hacc")
            scr = pp.tile([128, 1024], f32, tag="scr")
            qacc = pp.tile([M, 2], f32, tag="qacc")
            qacc2 = pp.tile([M, 1], f32, tag="qacc2")
            qtot = pp.tile([1, 1], f32, tag="qtot")
            wtot = pp.tile([1, 1], f32, tag="wtot")
            htot = pp.tile([1, 1], f32, tag="htot")

            # ---- input DMAs
            nc.sync.dma_start(out=inpS[:], in_=inp[:])
            nc.sync.dma_start(out=uS[:M, :], in_=uD[:])
            # duplicate U onto partitions 64:128 (for the b-pair stacking)
            nc.sync.dma_start(out=uS[M:128, :], in_=uS[:M, :])

            hp = inpS[:, OFF_HP : OFF_HP + 528]
            ep = inpS[:, OFF_EP : OFF_EP + CH]
            eM = inpS[:, OFF_E : OFF_E + 512]
            cv = inpS[:, OFF_CV : OFF_CV + 64]

            # rounded (f32r) copies of matmul constants
            nc.vector.tensor_copy(out=idR[:], in_=inpS[:, OFF_ID : OFF_ID + 128])
            nc.vector.tensor_copy(out=ksR[:], in_=inpS[:, OFF_KS : OFF_KS + 512])
            nc.vector.tensor_copy(out=ttR[:], in_=inpS[:, OFF_TT : OFF_TT + TPC])

            # ---- U normalization -> ustack (experts on partitions, dup halves)
            uview = uS[:].rearrange("p (d b) -> p d b", b=B)
            for bb in range(B):
                uscr = up.tile([128, D + 1], f32, tag="uscr", bufs=2)
                nc.scalar.activation(
                    out=uscr[:],
                    in_=uview[:, :, bb],
                    func=AF.Square,
                    accum_out=nsq[:, bb : bb + 1],
                )
            nc.scalar.activation(out=nrt[:], in_=nsq[:], func=AF.Sqrt)
            nc.vector.reciprocal(out=rinv[:], in_=nrt[:])
            for bb in range(8):
                nc.vector.tensor_scalar(
                    out=ustack[bb][:M, :],
                    in0=uview[:M, :D, bb],
                    scalar1=rinv[:M, bb : bb + 1],
                    scalar2=None,
                    op0=ALU.mult,
                )
                nc.vector.tensor_scalar(
                    out=ustack[bb][M:128, :],
                    in0=uview[M:128, :D, bb + 8],
                    scalar1=rinv[M:128, bb + 8 : bb + 9],
                    scalar2=None,
                    op0=ALU.mult,
                )

            # ================= phase A (+ U transposes), own PSUM scope
            with (
                tc.tile_pool(name="psA", bufs=2, space="PSUM") as ppsA,
                tc.tile_pool(name="psB", bufs=1, space="PSUM") as ppsB,
            ):
                # ---- Un^T tiles (d on partitions) via PE transpose
                for dch in range(4):
                    psT = ppsA.tile([128, 1024], f32r, space="PSUM", tag="psT")
                    for ab in range(B):
                        src = ustack[ab % 8]
                        rows = src[:M, :] if ab < 8 else src[M:128, :]
                        ident = idR[:M, :M] if ab < 8 else idR[M:128, M:128]
                        nc.tensor.transpose(
                            out=psT[:, 64 * ab : 64 * ab + 64],
                            in_=rows[:, 128 * dch : 128 * dch + 128],
                            identity=ident,
                        )
                    nc.scalar.copy(
                        out=unt[dch][:].rearrange("p (m b) -> p b m", b=B),
                        in_=psT[:].rearrange("p (b m) -> p b m", m=M),
                    )

                # ---- build S2 (+shifted dup) and counts
                psCnt = ppsB.tile([M, NS], f32, space="PSUM", tag="psCnt")
                hview = hp.rearrange("p (c b) -> p c b", b=B)
                p2cyc = [
                    pp.tile([128, 192], f32r, tag=f"p2_{i}", name=f"p2_{i}")
                    for i in range(3)
                ]
                for t in p2cyc:
                    nc.vector.memset(t[:, 0:64].bitcast(f32), 0.0)
                    nc.vector.memset(t[:, 128:192].bitcast(f32), 0.0)
                for c in range(CH):
                    p2 = p2cyc[c % 3]
                    nc.vector.tensor_scalar(
                        out=p2[:, 64:128],
                        in0=cv,
                        scalar1=ep[:, c : c + 1],
                        scalar2=None,
                        op0=ALU.is_equal,
                    )
                    dc = wp.tile([128, 512], f32r, tag="dc")
                    dsh = wp.tile([128, 512], f32r, tag="dsh")
                    nc.vector.tensor_tensor(
                        out=dc[:].rearrange("p (t b) -> p t b", b=B),
                        in0=eM.rearrange("p (t b) -> p t b", b=B),
                        in1=hview[:, c, :][:, None, :].to_broadcast((128, TPC, B)),
                        op=ALU.mult,
                    )
                    nc.gpsimd.tensor_tensor(
                        out=dsh[:].rearrange("p (t b) -> p t b", b=B),
                        in0=eM.rearrange("p (t b) -> p t b", b=B),
                        in1=hp[:, 16 * c + 8 : 16 * c + 24][:, None, :].to_broadcast(
                            (128, TPC, B)
                        ),
                        op=ALU.mult,
                    )
                    psA = ppsA.tile([128, 512], f32, space="PSUM", tag="psA")
                    nc.tensor.matmul(
                        psA[:], lhsT=p2[:, 64:192], rhs=dc[:], start=True, stop=False
                    )
                    nc.tensor.matmul(
                        psA[:], lhsT=p2[:, 0:128], rhs=dsh[:], start=False, stop=True
                    )
                    nc.tensor.matmul(
                        psCnt[:, TPC * c : TPC * (c + 1)],
                        lhsT=p2[:, 64:128],
                        rhs=ttR[:],
                        start=True,
                        stop=True,
                    )
                    nc.scalar.copy(out=s2d[:, 512 * c : 512 * (c + 1)], in_=psA[:])

                nc.scalar.copy(out=cntS[:], in_=psCnt[:])

            # ================= writes phase (needs all 8 PSUM banks)
            with tc.tile_pool(name="psW", bufs=8, space="PSUM") as ppsW:
                s2view = s2d[:].rearrange("p (t b) -> p t b", b=B)
                for dch in range(4):
                    for nch in range(2):
                        psW = ppsW.tile([128, 512], f32, space="PSUM", tag="psW")
                        for bb in range(8):
                            nc.tensor.matmul(
                                psW[:],
                                lhsT=ustack[bb][:, 128 * dch : 128 * (dch + 1)],
                                rhs=s2view[:, 512 * nch : 512 * (nch + 1), bb],
                                start=(bb == 0),
                                stop=(bb == 7),
                            )
                        nc.scalar.copy(
                            out=wtT[
                                :, 1024 * dch + 512 * nch : 1024 * dch + 512 * (nch + 1)
                            ],
                            in_=psW[:],
                        )
                    nc.sync.dma_start(
                        out=wtD[:, 1024 * dch : 1024 * (dch + 1)],
                        in_=wtT[:, 1024 * dch : 1024 * (dch + 1)].bitcast(f32),
                    )
                # ||writes||^2 and sum(h^2) (DVE fused square+reduce)
                for dch in range(4):
                    sl = wtT[:, 1024 * dch : 1024 * (dch + 1)]
                    nc.scalar.activation(
                        out=scr[:],
                        in_=sl,
                        func=AF.Square,
                        accum_out=wacc[:, dch : dch + 1],
                    )
                nc.scalar.activation(
                    out=scr[:, :512],
                    in_=inpS[:, OFF_HP : OFF_HP + 512],
                    func=AF.Square,
                    accum_out=hacc[:],
                )
                nc.vector.tensor_reduce(
                    out=wacc2[:], in_=wacc[:], axis=mybir.AxisListType.X, op=ALU.add
                )

            # ================= recon/loss phase
            with (
                tc.tile_pool(name="psR", bufs=2, space="PSUM") as ppsR,
                tc.tile_pool(name="psQ", bufs=2, space="PSUM") as ppsQ,
            ):
                psQ = [
                    ppsQ.tile([M, 512], f32, space="PSUM", tag=f"psQ{i}", name=f"psQ{i}")
                    for i in range(2)
                ]
                for nch in range(2):
                    for j in range(8):
                        psR = ppsR.tile([128, 512], f32, space="PSUM", tag="psR")
                        for dch in range(4):
                            nc.tensor.matmul(
                                psR[:],
                                lhsT=unt[dch][:, 128 * j : 128 * (j + 1)],
                                rhs=wtT[
                                    :,
                                    1024 * dch + 512 * nch : 1024 * dch + 512 * (nch + 1),
                                ],
                                start=(dch == 0),
                                stop=(dch == 3),
                            )
                        sqR = wp.tile([128, 512], f32r, tag="sqR")
                        nc.scalar.activation(out=sqR[:], in_=psR[:], func=AF.Square)
                        nc.tensor.matmul(
                            psQ[nch][:],
                            lhsT=ksR[:, 64 * j : 64 * (j + 1)],
                            rhs=sqR[:],
                            start=(j == 0),
                            stop=(j == 7),
                        )
                for nch in range(2):
                    nc.vector.scalar_tensor_tensor(
                        out=scr[:M, :512],
                        in0=psQ[nch][:],
                        scalar=1.0,
                        in1=cntS[:, 512 * nch : 512 * (nch + 1)],
                        op0=ALU.mult,
                        op1=ALU.mult,
                        accum_out=qacc[:, nch : nch + 1],
                    )
                nc.vector.tensor_reduce(
                    out=qacc2[:], in_=qacc[:], axis=mybir.AxisListType.X, op=ALU.add
                )
                # cross-partition reduce -> scalars (single packed op)
                acc3 = pp.tile([128, 4], f32, tag="acc3", name="acc3")
                tot3 = pp.tile([1, 4], f32, tag="tot3", name="tot3")
                nc.vector.memset(acc3[:], 0.0)
                nc.vector.tensor_copy(out=acc3[:M, 0:1], in_=qacc2[:])
                nc.vector.tensor_copy(out=acc3[:, 1:2], in_=wacc2[:])
                nc.vector.tensor_copy(out=acc3[:, 2:3], in_=hacc[:])
                nc.gpsimd.tensor_reduce(
                    out=tot3[:], in_=acc3[:], axis=mybir.AxisListType.C, op=ALU.add
                )
                nc.vector.memset(pl[:], 0.0)
                nc.vector.tensor_copy(out=pl[:, 0:4], in_=tot3[:])
                nc.sync.dma_start(out=plD[:], in_=pl[:])

    nc.compile()
    return nc


def _host_constants():
    p = np.arange(128)
    t = p // K
    E = np.zeros((128, 512), np.float32)
    for b in range(B):
        E[p, 16 * t + b] = 1.0
    TT = np.zeros((128, TPC), np.float32)
    TT[p, t] = 1.0
    CV = np.broadcast_to(np.arange(64, dtype=np.float32), (128, 64)).copy()
    ID = np.eye(128, dtype=np.float32)
    KS = np.zeros((128, 512), np.float32)
    q = np.arange(128)
    for j in range(8):
        KS[q, 64 * j + 8 * j + q // 16] = 1.0
    return E, TT, CV, ID, KS


_NC_CACHE = {}


def kernel(h_sparse, topk_idxs, U):
    h_sparse = np.asarray(h_sparse, dtype=np.float32)
    topk_idxs = np.asarray(topk_idxs)
    U = np.asarray(U, dtype=np.float32)

    if "nc" not in _NC_CACHE:
        _NC_CACHE["nc"] = build_nc()
    nc = _NC_CACHE["nc"]

    E, TT, CV, ID, KS = _host_constants()
    u_in = np.ascontiguousarray(U.reshape(M, UF))

    in_maps = []
    for i in range(NC):
        sl = slice(NS * i, NS * (i + 1))
        hs = h_sparse[sl].reshape(NPAIR, B)
        hp = np.zeros((128, 528), np.float32)
        hp[:, :512] = hs.reshape(CH, 128, B).transpose(1, 0, 2).reshape(128, 512)
        ep = (
            topk_idxs[sl].reshape(NPAIR).astype(np.float32).reshape(CH, 128).T.copy()
        )
        inp = np.zeros((128, INP_F), np.float32)
        inp[:, OFF_HP : OFF_HP + 528] = hp
        inp[:, OFF_EP : OFF_EP + CH] = ep
        inp[:, OFF_E : OFF_E + 512] = E
        inp[:, OFF_TT : OFF_TT + TPC] = TT
        inp[:, OFF_CV : OFF_CV + 64] = CV
        inp[:, OFF_ID : OFF_ID + 128] = ID
        inp[:, OFF_KS : OFF_KS + 512] = KS
        in_maps.append({"inp": inp, "u": u_in})

    br = run_bass_kernel_spmd(
        nc, in_maps, list(range(NC)), trace=bool(_NC_CACHE.get("trace"))
    )
    if _NC_CACHE.get("trace"):
        _NC_CACHE["last_exec_ns"] = br.exec_time_ns
        _NC_CACHE["last_profile"] = br.profile_json
    res = br.results

    writes = np.empty((N, D), np.float32)
    cr2 = w2 = h2 = 0.0
    for i in range(NC):
        wt2 = res[i]["wt"]  # (128, 4*NS)
        writes[NS * i : NS * (i + 1)] = (
            wt2.reshape(128, 4, NS).transpose(2, 1, 0).reshape(NS, D)
        )
        plv = res[i]["pl"][0]
        cr2 += float(plv[0])
        w2 += float(plv[1])
        h2 += float(plv[2])
    loss = (cr2 - 2.0 * w2 + h2) / float(N * K * B)
    return writes, np.float32(loss)
